# revision 33
# baseline (speedup 1.0000x reference)
"""GQA sliding-window attention (training path, no causal mask, no 1/sqrt(d)
scaling) on 8 Trainium2 NeuronCores.

Reference semantics (see original nn.Module):
  q = x@Wq+bq [b,s,16,64]; k,v = x@Wk+bk / x@Wv+bv [b,s,2,64]
  k,v zero-padded by 128 on both sides of s; query i attends padded
  positions [i, i+256) (i.e. global [i-128, i+128)); padded positions
  contribute score 0 (exp->1) and value 0. out = attn @ Wo + bo.

Sharding: batch x sequence. 8 shards = 2 batches x 4 chunks of 512 query
rows. Each core receives x^T for its 512 rows plus a 128-row halo on each
side (zero rows outside [0, 2048)), with an appended 0/1 validity row so
that K/V bias is only added at in-range positions (k = x@Wk + valid*bk).
Host gathers/concatenates per-core outputs; no collectives.

Per-core dataflow (all fp32; matmuls issued as float32r):
  xT -> qT [dk, s] / kT / vT projections (PE, contraction over 1024)
  vT transposed back to V [w, dk] via PE transpose; a ones-column is
  appended to V so each PV matmul also produces the softmax denominator.
  Scores computed transposed, S^T[w, q] = kT^T qT, per 128-wide kv chunk
  (6 chunks cover the 768 halo), q-window 384 per chunk.
  exp on ScalarE (one batched op per 3 chunks), band masking via
  GPSIMD affine_select triangles, PV accumulated over chunks into a
  [65, 512] PSUM tile (row 64 = denominator). Normalization is per head
  pair: 1/den via the fast DVE reciprocal straight off the PV PSUM row,
  broadcast across partitions with a tiny select matmul, multiplied into
  attnT right away so the Wo projection streams without a tail stall.

Input DMAs are consolidated into a handful of large strided transfers
issued in compute-priority order (wkv, x, wq halves, wo) on one queue so
the PE starts as soon as the first megabyte lands instead of after the
whole 15MB.
"""

import numpy as np

DIM = 1024
NH = 16  # query heads
G = 2  # kv heads
HD = 64  # head dim
W = 256  # window
HALF = 128
BATCH, SEQ = 2, 2048
NCORES = 8
SQ = 512  # query rows per core
SK = SQ + 2 * HALF  # 768 kv halo rows per core
KC = DIM // 128  # 8 contraction chunks
NJ = SK // 128  # 6 kv chunks
SP = 384  # score q-window width per kv chunk

# chunk j's PV accumulation window [lo, hi) in local q coords. Edge chunks
# 0/5 are widened to 256 so every PV matmul has N>=256 (1 cycle/row f32r);
# the widened region is zeroed by the same affine_select that cuts the
# triangle, so the extra columns contribute nothing.
PV_WIN = {0: (0, 256), 1: (0, 256), 2: (0, 384), 3: (128, 512), 4: (256, 512), 5: (256, 512)}
WS = {j: (0 if j < 3 else 128) for j in range(NJ)}
# PV issue order: j1 [0,256) and j4 [256,512) partition the PSUM zero
# region exactly, so every byte is written once before any accumulation
# (has_written zero-region semantics); stop on the last.
PV_ORDER = [1, 4, 0, 2, 3, 5]

_CACHE = {}


def _build_program(dbg=False):
    import concourse.bass as bass
    import concourse.mybir as mybir
    import concourse.tile as tile
    from concourse import bacc

    f32 = mybir.dt.float32
    f32r = mybir.dt.float32r

    nc = bacc.Bacc("TRN2", target_bir_lowering=False, debug=False, num_devices=NCORES)
    dbg_t = {}
    if dbg:
        for name, shape, dt_ in [
            ("dbg_denf", [128, SQ], f32), ("dbg_denr", [128, SQ], f32),
            ("dbg_attnT", [128, KC, SQ], f32),
        ]:
            dbg_t[name] = nc.declare_dram_parameter(name, shape, dt_, isOutput=True)

    # All big inputs are host-packed to the exact SBUF image ([partition,
    # chunk, col], contiguous) so each load is one fully-contiguous DMA:
    # 128 descriptors with multi-KB lines instead of thousands of 1KB ones.
    xaTp = nc.declare_dram_parameter("xaTp", [128, KC, SK], f32r, isOutput=False)
    xaug_d = nc.declare_dram_parameter("xaug", [1, SK], f32r, isOutput=False)
    wq0 = nc.declare_dram_parameter("wq0", [128, KC, 512], f32r, isOutput=False)
    wq1 = nc.declare_dram_parameter("wq1", [128, KC, 512], f32r, isOutput=False)
    wkp = nc.declare_dram_parameter("wkp", [128, KC, G * HD], f32r, isOutput=False)
    wvp = nc.declare_dram_parameter("wvp", [128, KC, G * HD], f32r, isOutput=False)
    wkv_aug_d = nc.declare_dram_parameter("wkv_aug", [1, 2 * G * HD], f32r, isOutput=False)
    selT = nc.declare_dram_parameter("selT", [128, 128], f32r, isOutput=False)
    wo = nc.declare_dram_parameter("wo", [128, KC, DIM], f32r, isOutput=False)
    bq = nc.declare_dram_parameter("bq", [DIM, 1], f32, isOutput=False)
    bo = nc.declare_dram_parameter("bo", [DIM, 1], f32, isOutput=False)
    identD = nc.declare_dram_parameter("ident", [128, 128], f32r, isOutput=False)
    ones2 = nc.declare_dram_parameter("ones2", [128, G], f32r, isOutput=False)
    yT = nc.declare_dram_parameter("yT", [DIM, SQ], f32, isOutput=True)

    def r(ap):
        return ap

    with tile.TileContext(nc) as tc:
        with (
            nc.allow_low_precision("fp32r (tf32) matmul inputs; accumulation stays fp32"),
            tc.tile_pool(name="wts", bufs=1) as wts,
            tc.tile_pool(name="sb", bufs=1) as sb,
            tc.tile_pool(name="pt", bufs=3) as ptp,
            tc.tile_pool(name="yst", bufs=2) as yst,
            tc.tile_pool(name="psA", bufs=3, space="PSUM") as psA,
            tc.tile_pool(name="pvP", bufs=2, space="PSUM") as pvP,
        ):
            # ---- big loads: one fully-contiguous DMA each on the sync
            # queue, issued in compute-priority order so completion is
            # progressive: wk -> x (2-chunk pieces, so the K projection
            # trails the stream tightly) -> wv -> wq halves -> wo.
            wk_sb = wts.tile([128, KC, G * HD], f32r, tag="wk")
            nc.sync.dma_start(out=wk_sb[:, :, :], in_=wkp[:, :, :])
            xT_sb = wts.tile([128, KC, SK], f32r, tag="xT")
            for c2 in range(4):
                nc.sync.dma_start(out=xT_sb[:, 2 * c2:2 * c2 + 2, :],
                                  in_=xaTp[:, 2 * c2:2 * c2 + 2, :])
            wv_sb = wts.tile([128, KC, G * HD], f32r, tag="wv")
            nc.sync.dma_start(out=wv_sb[:, :, :], in_=wvp[:, :, :])
            wq0_sb = wts.tile([128, KC, 512], f32r, tag="wq0")
            wq1_sb = wts.tile([128, KC, 512], f32r, tag="wq1")
            nc.sync.dma_start(out=wq0_sb[:, :, :], in_=wq0[:, :, :])
            nc.sync.dma_start(out=wq1_sb[:, :, :], in_=wq1[:, :, :])
            wo_sb = wts.tile([128, KC, DIM], f32r, tag="wo")
            nc.sync.dma_start(out=wo_sb[:, :, :], in_=wo[:, :, :])

            # ---- small constants, split over the gpsimd SWDGE and scalar
            # queues (issued after the big streams so their descriptors
            # don't clog the rings ahead of the weights)
            xaug = wts.tile([1, SK], f32r, tag="xaug")
            nc.gpsimd.dma_start(out=xaug[:, :], in_=xaug_d[:, :])
            wkv_aug = wts.tile([1, 2 * G * HD], f32r, tag="wkvaug")
            nc.gpsimd.dma_start(out=wkv_aug[:, :], in_=wkv_aug_d[:, :])
            bq_sb = wts.tile([128, KC], f32, tag="bq")
            bo_sb = wts.tile([128, KC], f32, tag="bo")
            nc.gpsimd.dma_start(
                out=bq_sb[:, :], in_=bq.rearrange("(a p) c -> p (a c)", p=128))
            nc.scalar.dma_start(
                out=bo_sb[:, :], in_=bo.rearrange("(a p) c -> p (a c)", p=128))
            ident = wts.tile([128, 128], f32r, tag="ident")
            nc.scalar.dma_start(out=ident[:, :], in_=identD[:, :])
            ones_sb = wts.tile([128, G], f32r, tag="ones")
            nc.scalar.dma_start(out=ones_sb[:, :], in_=ones2[:, :])
            selT_sb = wts.tile([128, 128], f32r, tag="selT")
            nc.scalar.dma_start(out=selT_sb[:, :], in_=selT[:, :])

            # ---- persistent intermediates ----
            qT_sb = sb.tile([128, KC, SQ], f32r, tag="qT")     # [dk(2 heads), dd, q]
            kT_sb = sb.tile([128, SK], f32r, tag="kT")         # [dk(2 groups), w]
            vT_sb = sb.tile([128, SK], f32r, tag="vT")
            vt_t = [
                sb.tile([128, G, HD + 1], f32r, tag=f"vt{j}", name=f"vt{j}")
                for j in range(NJ)
            ]
            attnT = sb.tile([128, KC, SQ], f32r, tag="attnT")  # [dk(2 heads), pair, q]
            # reciprocal denominators. The custom-DVE fast reciprocal only
            # works from/to SBUF at base partition 0 on hardware (CoreSim
            # idealizes other bases): copy the PSUM den row -> den_s (base
            # 0), approx-reciprocal -> den_t (base 0). A gpsimd
            # partition_broadcast then fans each head's row across 64
            # partitions (rb tiles, also written at base 0) for the
            # in-place normalization multiply.
            den_s = sb.tile([1, G, SQ], f32, tag="dens")
            den_t = sb.tile([1, G, SQ], f32, tag="dent")
            den_f = sb.tile([128, SQ], f32, tag="denf")
            den_r = sb.tile([128, SQ], f32r, tag="denr")
            # memset can't target f32r; bounce the 1.0 fill through den_f
            nc.vector.memset(den_f[:, :], 1.0)
            nc.vector.tensor_copy(den_r[:, :], den_f[:, :])

            # ---- K/V projections over the full 768 halo (+ aug bias row) ----
            for (c0, wsb, dst) in ((0, wk_sb, kT_sb), (G * HD, wv_sb, vT_sb)):
                for h2 in range(2):
                    ps = psA.tile([128, 2, 512], f32, tag="ps")
                    out = ps[:, 0, 0:SP]
                    sl = slice(h2 * SP, (h2 + 1) * SP)
                    for kc in range(KC):
                        nc.tensor.matmul(
                            out, r(wsb[:, kc, :]), r(xT_sb[:, kc, sl]),
                            start=(kc == 0), stop=False,
                        )
                    nc.tensor.matmul(out, r(wkv_aug[:, c0:c0 + G * HD]), r(xaug[:, sl]),
                                     start=False, stop=True)
                    nc.vector.tensor_copy(dst[:, sl], out)

            # ---- V back to natural layout [w, dk], ones column appended ----
            for j in range(NJ):
                ps = psA.tile([128, 2, 512], f32r, tag="ps", name=f"pstr{j}")
                out = ps[:, 0, 0:128]
                nc.tensor.transpose(out, vT_sb[:, j * 128:(j + 1) * 128], ident)
                nc.vector.tensor_copy(
                    vt_t[j][:, :, 0:HD],
                    out.rearrange("p (g d) -> p g d", g=G),
                )
                nc.vector.tensor_copy(vt_t[j][:, :, HD:HD + 1], ones_sb[:, :])

            # ---- attention per head, normalization per pair ----
            # Host permutes Wq columns so q dd-block p holds head p (group 0)
            # in rows 0:64 and head p+8 (group 1) in rows 64:128 — score
            # matmul operands then share a base partition with kT's groups.
            def q_proj(dd):
                # Q projection block dd: qT[dd] = (Wq^T x^T)[dd] + bq.
                # Emitted just-in-time inside the attention loop so the PE
                # stream stays dense (512-wide projection matmuls fill the
                # gaps between attention groups and keep the clock un-gated).
                # Uses the 1-bank pvP pool so the score psum pool keeps all
                # three of its buffers for the psc tiles.
                ps = pvP.tile([128, 512], f32, tag="pv", name=f"psq{dd}")
                wsb = wq0_sb if dd < 4 else wq1_sb
                for kc in range(KC):
                    nc.tensor.matmul(
                        ps[:, :], r(wsb[:, kc, (dd % 4) * 128:(dd % 4 + 1) * 128]),
                        r(xT_sb[:, kc, HALF:HALF + SQ]),
                        start=(kc == 0), stop=(kc == KC - 1),
                    )
                nc.scalar.activation(
                    qT_sb[:, dd, :], ps[:, :], mybir.ActivationFunctionType.Identity,
                    bias=bq_sb[:, dd:dd + 1],
                )

            def norm_pair(p):
                # broadcast 1/den across partitions (rows 0:64 <- g0 head,
                # 64:128 <- g1 head) and normalize attnT[:, p, :] in place.
                b0 = 64 * (p % 2)
                ps = psA.tile([128, 2, 512], f32, tag="ps", name=f"psrb{p}")
                rb = ps[:, 0, :]
                nc.tensor.matmul(
                    rb, r(selT_sb[b0:b0 + 33, :]),
                    r(den_r[b0:b0 + 33, :]),
                    start=True, stop=True,
                )
                nc.vector.tensor_mul(attnT[:, p, :], attnT[:, p, :], rb)

            def emit_scores(p, gg):
                h = p + 8 * gg
                qT_h = qT_sb[64 * gg:64 * gg + 64, p, :]
                psc = [
                    psA.tile([128, 2, 512], f32, tag="ps", name=f"psc{h}_{i}")
                    for i in range(3)
                ]
                for j in range(NJ):
                    ws = WS[j]
                    nc.tensor.matmul(
                        psc[j // 2][:, j % 2, 0:SP],
                        r(kT_sb[64 * gg:64 * gg + 64, j * 128:(j + 1) * 128]),
                        r(qT_h[:, ws:ws + SP]),
                        start=True, stop=True,
                    )
                return psc

            def emit_softmax(psc):
                pt = ptp.tile([128, NJ, SP], f32r, tag="pt")
                # exp only the columns the PV windows read: thirds cover
                # chunk pairs (0,1): cols [0,256), (2,3): [0,384),
                # (4,5): [128,384). Emission order 0,2,1 matches PV_ORDER's
                # chunk consumption (j1 then j4) so the first PV matmuls
                # aren't stuck behind an exp they don't need.
                for c3 in (0, 2, 1):
                    elo, ehi = ((0, 256), (0, SP), (128, SP))[c3]
                    nc.scalar.activation(pt[:, 2 * c3:2 * c3 + 2, elo:ehi],
                                         psc[c3][:, :, elo:ehi],
                                         mybir.ActivationFunctionType.Exp)

                # band masking: keep iff 0 <= (128j + ww) - q < 256.
                # The upper-bound select on chunk 0 spans cols [0,256) (its
                # widened PV window): cols >= 128 fail the condition for
                # every partition and are filled with 0. Likewise the
                # lower-bound select on chunk 5. Chunks masked in PV_ORDER
                # so the serial gpsimd stream feeds the PV matmuls in their
                # issue order.
                for j in PV_ORDER:
                    lo, hi = PV_WIN[j]
                    ws = WS[j]
                    cb = lo
                    while cb < hi:
                        wdt = 256 if (j in (0, 5) and cb == (0 if j == 0 else 256)) else 128
                        c0 = cb - ws
                        region = pt[:, j, c0:c0 + wdt]
                        if cb > 128 * (j - 1):  # upper bound: q <= 128j + ww
                            nc.gpsimd.affine_select(
                                out=region, in_=region,
                                compare_op=mybir.AluOpType.is_ge, fill=0.0,
                                base=128 * j - cb, channel_multiplier=1,
                                pattern=[[-1, wdt]],
                            )
                        elif cb < 128 * (j - 1):  # lower: q > 128j + ww - 256
                            nc.gpsimd.affine_select(
                                out=region, in_=region,
                                compare_op=mybir.AluOpType.is_ge, fill=0.0,
                                base=cb - 128 * j + 255, channel_multiplier=-1,
                                pattern=[[1, wdt]],
                            )
                        cb += wdt
                return pt

            def emit_pv(p, gg, pt):
                qrow = 64 * gg
                pv = pvP.tile([128, 512], f32, tag="pv")
                for j in PV_ORDER:
                    lo, hi = PV_WIN[j]
                    ws = WS[j]
                    nc.tensor.matmul(
                        pv[0:HD + 1, lo:hi],
                        r(vt_t[j][:, gg, :]),
                        r(pt[:, j, lo - ws:hi - ws]),
                        start=(j == PV_ORDER[0]), stop=(j == PV_ORDER[-1]),
                    )
                if (p, gg) == (KC - 1, G - 1):
                    # last pair: the reciprocal chain is the tail's critical
                    # path; push the two copies to the (now idle) scalar
                    # engine so the DVE only runs the reciprocal + rounding.
                    nc.scalar.activation(attnT[qrow:qrow + 64, p, :], pv[0:HD, :],
                                         mybir.ActivationFunctionType.Copy)
                    nc.scalar.activation(den_s[:, gg, :], pv[HD:HD + 1, :],
                                         mybir.ActivationFunctionType.Copy)
                else:
                    nc.vector.tensor_copy(attnT[qrow:qrow + 64, p, :], pv[0:HD, :])
                    nc.vector.tensor_copy(den_s[:, gg, :], pv[HD:HD + 1, :])
                nc.vector.reciprocal_approx_fast(
                    out=den_t[:, gg, :], in_=den_s[:, gg, :],
                )
                drow = 64 * (p % 2) + 32 * gg
                nc.vector.tensor_copy(den_r[drow:drow + 1, :], den_t[:, gg, :])

            # Per pair, the PE stream is: scores_g0, qproj fill, scores_g1,
            # rb fill, PV_g0, PV_g1 — both heads' exp+mask latency hides
            # behind the projection/score matmuls, so the PE never waits on
            # the scalar/gpsimd softmax chain and its clock stays ramped.
            q_proj(0)
            for p in range(KC):
                psc0 = emit_scores(p, 0)
                pt0 = emit_softmax(psc0)
                if p < KC - 1:
                    q_proj(p + 1)
                psc1 = emit_scores(p, 1)
                if p > 0:
                    # previous pair's denominators are ready by now (their
                    # reciprocals overlapped this pair's score matmuls)
                    norm_pair(p - 1)
                pt1 = emit_softmax(psc1)
                emit_pv(p, 0, pt0)
                emit_pv(p, 1, pt1)
            norm_pair(KC - 1)

            if dbg:
                nc.sync.dma_start(out=dbg_t["dbg_denf"][:, :], in_=den_f[:, :])
                nc.gpsimd.dma_start(out=dbg_t["dbg_denr"][:, :], in_=den_r[:, :])
                nc.gpsimd.dma_start(out=dbg_t["dbg_attnT"][:, :, :], in_=attnT[:, :, :])

            # ---- output projection ----
            for do in range(KC):
                ps = psA.tile([128, 2, 512], f32, tag="ps")
                out = ps[:, 0, :]
                for p in range(KC):
                    nc.tensor.matmul(
                        out, r(wo_sb[:, p, do * 128:(do + 1) * 128]),
                        r(attnT[:, p, :]),
                        start=(p == 0), stop=(p == KC - 1),
                    )
                yt = yst.tile([128, SQ], f32, tag="yt")
                nc.scalar.activation(yt, out, mybir.ActivationFunctionType.Identity,
                                     bias=bo_sb[:, do:do + 1])
                eng = nc.sync if do % 2 == 0 else nc.scalar
                eng.dma_start(out=yT[do * 128:(do + 1) * 128, :], in_=yt[:, :])

    nc.finalize()
    return nc


def get_program():
    if "nc" not in _CACHE:
        _CACHE["nc"] = _build_program()
    return _CACHE["nc"]


def _pack(w):
    """[1024, C] weight -> SBUF image [128 partitions, KC chunks, C]."""
    return np.ascontiguousarray(w.reshape(KC, 128, -1).transpose(1, 0, 2))


def make_in_maps(x, Wq, bq, Wk, bk, Wv, bv, Wo, bo):
    """Host-side sharding: per-core input dicts."""
    x = np.ascontiguousarray(np.asarray(x, np.float32))
    wkv_aug = np.concatenate([np.asarray(bk, np.float32), np.asarray(bv, np.float32)])
    # head permutation: device column-block p holds [head p | head p+8]
    # (so each q dd-block pairs a group-0 head with a group-1 head at
    # matching base partitions). perm maps device attn-dim -> original dim.
    perm = np.empty(DIM, np.int64)
    for p in range(8):
        perm[128 * p:128 * p + 64] = np.arange(64 * p, 64 * p + 64)
        perm[128 * p + 64:128 * p + 128] = np.arange(64 * (p + 8), 64 * (p + 8) + 64)
    wq_p = np.asarray(Wq, np.float32)[:, perm]
    # select weight: pair p (b0 = 64*(p%2)) broadcasts den_r row b0 to
    # partitions 0:64 and row b0+32 to partitions 64:128.
    selT = np.zeros((128, 128), np.float32)
    selT[0, :64] = 1.0
    selT[32, 64:] = 1.0
    selT[64, :64] = 1.0
    selT[96, 64:] = 1.0
    common = {
        "wq0": _pack(wq_p[:, 0:512]),
        "wq1": _pack(wq_p[:, 512:1024]),
        "wkp": _pack(np.asarray(Wk, np.float32)),
        "wvp": _pack(np.asarray(Wv, np.float32)),
        "wkv_aug": np.ascontiguousarray(wkv_aug.reshape(1, 2 * G * HD)),
        "selT": selT,
        "wo": _pack(np.asarray(Wo, np.float32)[perm, :]),
        "bq": np.ascontiguousarray(np.asarray(bq, np.float32)[perm].reshape(DIM, 1)),
        "bo": np.ascontiguousarray(np.asarray(bo, np.float32).reshape(DIM, 1)),
        "ident": np.eye(128, dtype=np.float32),
        "ones2": np.ones((128, G), np.float32),
    }
    in_maps = []
    for c in range(NCORES):
        b, t = divmod(c, NCORES // BATCH)
        s0 = SQ * t
        xa = np.zeros((SK, DIM + 1), np.float32)
        lo, hi = max(0, s0 - HALF), min(SEQ, s0 + SQ + HALF)
        xa[lo - (s0 - HALF):hi - (s0 - HALF), :DIM] = x[b, lo:hi]
        xa[lo - (s0 - HALF):hi - (s0 - HALF), DIM] = 1.0
        xaT = xa.T  # [1025, 768]
        in_maps.append({
            "xaTp": _pack(xaT[0:DIM, :]),
            "xaug": np.ascontiguousarray(xaT[DIM:DIM + 1, :]),
            **common,
        })
    return in_maps


def assemble_output(results):
    y = np.empty((BATCH, SEQ, DIM), np.float32)
    for c in range(NCORES):
        b, t = divmod(c, NCORES // BATCH)
        y[b, SQ * t:SQ * (t + 1), :] = results[c]["yT"].T
    return y


def kernel(**inputs):
    from concourse.bass_utils import run_bass_kernel_spmd

    nc = get_program()
    in_maps = make_in_maps(**inputs)
    last_err = None
    for _ in range(3):  # retry: transient NRT device wedges recover on rerun
        try:
            res = run_bass_kernel_spmd(nc, in_maps, list(range(NCORES)))
            return assemble_output(res.results)
        except Exception as e:  # noqa: BLE001
            last_err = e
    raise last_err


if __name__ == "__main__":
    import reference

    inputs = reference.setup_inputs()
    expected = np.asarray(reference.reference(**inputs))
    actual = kernel(**{k: np.asarray(v) for k, v in inputs.items()})
    rel = np.linalg.norm(actual - expected) / np.linalg.norm(expected)
    print("rel err:", rel)


# revision 42
# speedup vs baseline: 1.1736x; 1.1736x over previous
"""GQA sliding-window attention (training path, no causal mask, no 1/sqrt(d)
scaling) on 8 Trainium2 NeuronCores.

Reference semantics (see original nn.Module):
  q = x@Wq+bq [b,s,16,64]; k,v = x@Wk+bk / x@Wv+bv [b,s,2,64]
  k,v zero-padded by 128 on both sides of s; query i attends padded
  positions [i, i+256) (i.e. global [i-128, i+128)); padded positions
  contribute score 0 (exp->1) and value 0. out = attn @ Wo + bo.

Sharding: batch x sequence. 8 shards = 2 batches x 4 chunks of 512 query
rows. Each core receives x^T for its 512 rows plus a 128-row halo on each
side (zero rows outside [0, 2048)), with an appended 0/1 validity row so
that K/V bias is only added at in-range positions (k = x@Wk + valid*bk).
Host gathers/concatenates per-core outputs; no collectives.

Per-core dataflow (all fp32; matmuls issued as float32r):
  xT -> qT [dk, s] / kT / vT projections (PE, contraction over 1024)
  vT transposed back to V [w, dk] via PE transpose; a ones-column is
  appended to V so each PV matmul also produces the softmax denominator.
  Scores computed transposed, S^T[w, q] = kT^T qT, per 128-wide kv chunk
  (6 chunks cover the 768 halo), q-window 384 per chunk.
  exp on ScalarE (one batched op per 3 chunks), band masking via
  GPSIMD affine_select triangles, PV accumulated over chunks into a
  [65, 512] PSUM tile (row 64 = denominator). Normalization is per head
  pair: 1/den via the fast DVE reciprocal straight off the PV PSUM row,
  broadcast across partitions with a tiny select matmul, multiplied into
  attnT right away so the Wo projection streams without a tail stall.

Input DMAs are consolidated into a handful of large strided transfers
issued in compute-priority order (wkv, x, wq halves, wo) on one queue so
the PE starts as soon as the first megabyte lands instead of after the
whole 15MB.
"""

import numpy as np

DIM = 1024
NH = 16  # query heads
G = 2  # kv heads
HD = 64  # head dim
W = 256  # window
HALF = 128
BATCH, SEQ = 2, 2048
NCORES = 8
SQ = 512  # query rows per core
SK = SQ + 2 * HALF  # 768 kv halo rows per core
KC = DIM // 128  # 8 contraction chunks
NJ = SK // 128  # 6 kv chunks
SP = 384  # score q-window width per kv chunk

# chunk j's PV accumulation window [lo, hi) in local q coords. Edge chunks
# 0/5 are widened to 256 so every PV matmul has N>=256 (1 cycle/row f32r);
# the widened region is zeroed by the same affine_select that cuts the
# triangle, so the extra columns contribute nothing.
PV_WIN = {0: (0, 256), 1: (0, 256), 2: (0, 384), 3: (128, 512), 4: (256, 512), 5: (256, 512)}
WS = {j: (0 if j < 3 else 128) for j in range(NJ)}
# PV issue order: j1 [0,256) and j4 [256,512) partition the PSUM zero
# region exactly, so every byte is written once before any accumulation
# (has_written zero-region semantics); stop on the last.
PV_ORDER = [1, 4, 0, 2, 3, 5]

_CACHE = {}


def _build_program(dbg=False):
    import concourse.bass as bass
    import concourse.mybir as mybir
    import concourse.tile as tile
    from concourse import bacc

    f32 = mybir.dt.float32
    f32r = mybir.dt.float32r

    nc = bacc.Bacc("TRN2", target_bir_lowering=False, debug=False, num_devices=NCORES)
    dbg_t = {}
    if dbg:
        for name, shape, dt_ in [
            ("dbg_denf", [128, SQ], f32), ("dbg_denr", [128, SQ], f32),
            ("dbg_attnT", [128, KC, SQ], f32),
        ]:
            dbg_t[name] = nc.declare_dram_parameter(name, shape, dt_, isOutput=True)

    # All big inputs are host-packed to the exact SBUF image ([partition,
    # chunk, col], contiguous) so each load is one fully-contiguous DMA:
    # 128 descriptors with multi-KB lines instead of thousands of 1KB ones.
    xaTp = nc.declare_dram_parameter("xaTp", [128, KC, SK], f32r, isOutput=False)
    xaug_d = nc.declare_dram_parameter("xaug", [1, SK], f32r, isOutput=False)
    wqd0 = nc.declare_dram_parameter("wqd0", [128, KC, 128], f32r, isOutput=False)
    wq0 = nc.declare_dram_parameter("wq0", [128, KC, 384], f32r, isOutput=False)
    wq1 = nc.declare_dram_parameter("wq1", [128, KC, 512], f32r, isOutput=False)
    wkp = nc.declare_dram_parameter("wkp", [128, KC, G * HD], f32r, isOutput=False)
    wvp = nc.declare_dram_parameter("wvp", [128, KC, G * HD], f32r, isOutput=False)
    wkv_aug_d = nc.declare_dram_parameter("wkv_aug", [1, 2 * G * HD], f32r, isOutput=False)
    selT = nc.declare_dram_parameter("selT", [128, 128], f32r, isOutput=False)
    wo = nc.declare_dram_parameter("wo", [128, KC, DIM], f32r, isOutput=False)
    bq = nc.declare_dram_parameter("bq", [DIM, 1], f32, isOutput=False)
    bo = nc.declare_dram_parameter("bo", [DIM, 1], f32, isOutput=False)
    identD = nc.declare_dram_parameter("ident", [128, 128], f32r, isOutput=False)
    ones2 = nc.declare_dram_parameter("ones2", [128, G], f32r, isOutput=False)
    yT = nc.declare_dram_parameter("yT", [DIM, SQ], f32, isOutput=True)

    def r(ap):
        return ap

    with tile.TileContext(nc) as tc:
        with (
            nc.allow_low_precision("fp32r (tf32) matmul inputs; accumulation stays fp32"),
            tc.tile_pool(name="wts", bufs=1) as wts,
            tc.tile_pool(name="sb", bufs=1) as sb,
            tc.tile_pool(name="pt", bufs=3) as ptp,
            tc.tile_pool(name="yst", bufs=2) as yst,
            tc.tile_pool(name="psA", bufs=3, space="PSUM") as psA,
            tc.tile_pool(name="pvP", bufs=2, space="PSUM") as pvP,
        ):
            # ---- big loads: one fully-contiguous DMA each on the sync
            # queue, issued in compute-priority order so completion is
            # progressive: wk -> x (2-chunk pieces, so the K projection
            # trails the stream tightly) -> wv -> wq halves -> wo.
            wk_sb = wts.tile([128, KC, G * HD], f32r, tag="wk")
            nc.sync.dma_start(out=wk_sb[:, :, :], in_=wkp[:, :, :])
            xT_sb = wts.tile([128, KC, SK], f32r, tag="xT")
            for (a, b) in ((0, 1), (1, 2), (2, 4), (4, 6), (6, 8)):
                nc.sync.dma_start(out=xT_sb[:, a:b, :], in_=xaTp[:, a:b, :])
            wv_sb = wts.tile([128, KC, G * HD], f32r, tag="wv")
            nc.sync.dma_start(out=wv_sb[:, :, :], in_=wvp[:, :, :])
            # wq split so q_proj(0) only waits on its own 0.5MB column block
            wqd0_sb = wts.tile([128, KC, 128], f32r, tag="wqd0")
            nc.sync.dma_start(out=wqd0_sb[:, :, :], in_=wqd0[:, :, :])
            wq0_sb = wts.tile([128, KC, 384], f32r, tag="wq0")
            wq1_sb = wts.tile([128, KC, 512], f32r, tag="wq1")
            nc.sync.dma_start(out=wq0_sb[:, :, :], in_=wq0[:, :, :])
            nc.sync.dma_start(out=wq1_sb[:, :, :], in_=wq1[:, :, :])
            wo_sb = wts.tile([128, KC, DIM], f32r, tag="wo")
            nc.sync.dma_start(out=wo_sb[:, :, :], in_=wo[:, :, :])

            # ---- small constants, split over the gpsimd SWDGE and scalar
            # queues (issued after the big streams so their descriptors
            # don't clog the rings ahead of the weights)
            xaug = wts.tile([1, SK], f32r, tag="xaug")
            nc.gpsimd.dma_start(out=xaug[:, :], in_=xaug_d[:, :])
            wkv_aug = wts.tile([1, 2 * G * HD], f32r, tag="wkvaug")
            nc.gpsimd.dma_start(out=wkv_aug[:, :], in_=wkv_aug_d[:, :])
            bq_sb = wts.tile([128, KC], f32, tag="bq")
            bo_sb = wts.tile([128, KC], f32, tag="bo")
            nc.gpsimd.dma_start(
                out=bq_sb[:, :], in_=bq.rearrange("(a p) c -> p (a c)", p=128))
            nc.scalar.dma_start(
                out=bo_sb[:, :], in_=bo.rearrange("(a p) c -> p (a c)", p=128))
            ident = wts.tile([128, 128], f32r, tag="ident")
            nc.scalar.dma_start(out=ident[:, :], in_=identD[:, :])
            ones_sb = wts.tile([128, G], f32r, tag="ones")
            nc.scalar.dma_start(out=ones_sb[:, :], in_=ones2[:, :])
            selT_sb = wts.tile([128, 128], f32r, tag="selT")
            nc.scalar.dma_start(out=selT_sb[:, :], in_=selT[:, :])

            # ---- persistent intermediates ----
            qT_sb = sb.tile([128, KC, SQ], f32r, tag="qT")     # [dk(2 heads), dd, q]
            kT_sb = sb.tile([128, SK], f32r, tag="kT")         # [dk(2 groups), w]
            vT_sb = sb.tile([128, SK], f32r, tag="vT")
            vt_t = [
                sb.tile([128, G, HD + 1], f32r, tag=f"vt{j}", name=f"vt{j}")
                for j in range(NJ)
            ]
            attnT = sb.tile([128, KC, SQ], f32r, tag="attnT")  # [dk(2 heads), pair, q]
            # reciprocal denominators. The custom-DVE fast reciprocal only
            # works from/to SBUF at base partition 0 on hardware (CoreSim
            # idealizes other bases): copy the PSUM den row -> den_s (base
            # 0), approx-reciprocal -> den_t (base 0). A gpsimd
            # partition_broadcast then fans each head's row across 64
            # partitions (rb tiles, also written at base 0) for the
            # in-place normalization multiply.
            den_s = sb.tile([1, G, SQ], f32, tag="dens")
            den_t = sb.tile([1, G, SQ], f32, tag="dent")
            den_f = sb.tile([128, SQ], f32, tag="denf")
            den_r = sb.tile([128, SQ], f32r, tag="denr")
            # memset can't target f32r; bounce the 1.0 fill through den_f
            nc.vector.memset(den_f[:, :], 1.0)
            nc.vector.tensor_copy(den_r[:, :], den_f[:, :])

            # ---- K/V projections over the full 768 halo (+ aug bias row) ----
            for (c0, wsb, dst) in ((0, wk_sb, kT_sb), (G * HD, wv_sb, vT_sb)):
                for h2 in range(2):
                    ps = psA.tile([128, 2, 512], f32, tag="ps")
                    out = ps[:, 0, 0:SP]
                    sl = slice(h2 * SP, (h2 + 1) * SP)
                    for kc in range(KC):
                        nc.tensor.matmul(
                            out, r(wsb[:, kc, :]), r(xT_sb[:, kc, sl]),
                            start=(kc == 0), stop=False,
                        )
                    nc.tensor.matmul(out, r(wkv_aug[:, c0:c0 + G * HD]), r(xaug[:, sl]),
                                     start=False, stop=True)
                    nc.vector.tensor_copy(dst[:, sl], out)

            # ---- V back to natural layout [w, dk], ones column appended ----
            for j in range(NJ):
                ps = psA.tile([128, 2, 512], f32r, tag="ps", name=f"pstr{j}")
                out = ps[:, 0, 0:128]
                nc.tensor.transpose(out, vT_sb[:, j * 128:(j + 1) * 128], ident)
                nc.vector.tensor_copy(
                    vt_t[j][:, :, 0:HD],
                    out.rearrange("p (g d) -> p g d", g=G),
                )
                nc.vector.tensor_copy(vt_t[j][:, :, HD:HD + 1], ones_sb[:, :])

            # ---- attention per head, normalization per pair ----
            # Host permutes Wq columns so q dd-block p holds head p (group 0)
            # in rows 0:64 and head p+8 (group 1) in rows 64:128 — score
            # matmul operands then share a base partition with kT's groups.
            def q_proj(dd):
                # Q projection block dd: qT[dd] = (Wq^T x^T)[dd] + bq.
                # Emitted just-in-time inside the attention loop so the PE
                # stream stays dense (512-wide projection matmuls fill the
                # gaps between attention groups and keep the clock un-gated).
                # Uses the 1-bank pvP pool so the score psum pool keeps all
                # three of its buffers for the psc tiles.
                ps = pvP.tile([128, 512], f32, tag="pv", name=f"psq{dd}")
                if dd == 0:
                    wsb, c0 = wqd0_sb, 0
                elif dd < 4:
                    wsb, c0 = wq0_sb, (dd - 1) * 128
                else:
                    wsb, c0 = wq1_sb, (dd - 4) * 128
                for kc in range(KC):
                    nc.tensor.matmul(
                        ps[:, :], r(wsb[:, kc, c0:c0 + 128]),
                        r(xT_sb[:, kc, HALF:HALF + SQ]),
                        start=(kc == 0), stop=(kc == KC - 1),
                    )
                nc.scalar.activation(
                    qT_sb[:, dd, :], ps[:, :], mybir.ActivationFunctionType.Identity,
                    bias=bq_sb[:, dd:dd + 1],
                )

            def norm_pair(p):
                # broadcast 1/den across partitions (rows 0:64 <- g0 head,
                # 64:128 <- g1 head) and normalize attnT[:, p, :] in place.
                b0 = 64 * (p % 2)
                ps = psA.tile([128, 2, 512], f32, tag="ps", name=f"psrb{p}")
                rb = ps[:, 0, :]
                nc.tensor.matmul(
                    rb, r(selT_sb[b0:b0 + 33, :]),
                    r(den_r[b0:b0 + 33, :]),
                    start=True, stop=True,
                )
                nc.vector.tensor_mul(attnT[:, p, :], attnT[:, p, :], rb)

            def emit_scores(p, gg):
                h = p + 8 * gg
                qT_h = qT_sb[64 * gg:64 * gg + 64, p, :]
                psc = [
                    psA.tile([128, 2, 512], f32, tag="ps", name=f"psc{h}_{i}")
                    for i in range(3)
                ]
                for j in range(NJ):
                    ws = WS[j]
                    nc.tensor.matmul(
                        psc[j // 2][:, j % 2, 0:SP],
                        r(kT_sb[64 * gg:64 * gg + 64, j * 128:(j + 1) * 128]),
                        r(qT_h[:, ws:ws + SP]),
                        start=True, stop=True,
                    )
                return psc

            def emit_softmax(psc):
                pt = ptp.tile([128, NJ, SP], f32r, tag="pt")
                # exp only the columns the PV windows read: thirds cover
                # chunk pairs (0,1): cols [0,256), (2,3): [0,384),
                # (4,5): [128,384). Emission order 0,2,1 matches PV_ORDER's
                # chunk consumption (j1 then j4) so the first PV matmuls
                # aren't stuck behind an exp they don't need.
                for c3 in (0, 2, 1):
                    elo, ehi = ((0, 256), (0, SP), (128, SP))[c3]
                    nc.scalar.activation(pt[:, 2 * c3:2 * c3 + 2, elo:ehi],
                                         psc[c3][:, :, elo:ehi],
                                         mybir.ActivationFunctionType.Exp)

                # band masking: keep iff 0 <= (128j + ww) - q < 256.
                # The upper-bound select on chunk 0 spans cols [0,256) (its
                # widened PV window): cols >= 128 fail the condition for
                # every partition and are filled with 0. Likewise the
                # lower-bound select on chunk 5. Chunks masked in PV_ORDER
                # so the serial gpsimd stream feeds the PV matmuls in their
                # issue order.
                for j in PV_ORDER:
                    lo, hi = PV_WIN[j]
                    ws = WS[j]
                    cb = lo
                    while cb < hi:
                        wdt = 256 if (j in (0, 5) and cb == (0 if j == 0 else 256)) else 128
                        c0 = cb - ws
                        region = pt[:, j, c0:c0 + wdt]
                        if cb > 128 * (j - 1):  # upper bound: q <= 128j + ww
                            nc.gpsimd.affine_select(
                                out=region, in_=region,
                                compare_op=mybir.AluOpType.is_ge, fill=0.0,
                                base=128 * j - cb, channel_multiplier=1,
                                pattern=[[-1, wdt]],
                            )
                        elif cb < 128 * (j - 1):  # lower: q > 128j + ww - 256
                            nc.gpsimd.affine_select(
                                out=region, in_=region,
                                compare_op=mybir.AluOpType.is_ge, fill=0.0,
                                base=cb - 128 * j + 255, channel_multiplier=-1,
                                pattern=[[1, wdt]],
                            )
                        cb += wdt
                return pt

            def emit_pv(p, gg, pt):
                qrow = 64 * gg
                pv = pvP.tile([128, 512], f32, tag="pv")
                for j in PV_ORDER:
                    lo, hi = PV_WIN[j]
                    ws = WS[j]
                    nc.tensor.matmul(
                        pv[0:HD + 1, lo:hi],
                        r(vt_t[j][:, gg, :]),
                        r(pt[:, j, lo - ws:hi - ws]),
                        start=(j == PV_ORDER[0]), stop=(j == PV_ORDER[-1]),
                    )
                if (p, gg) == (KC - 1, G - 1):
                    # last pair: the reciprocal chain is the tail's critical
                    # path; push the two copies to the (now idle) scalar
                    # engine so the DVE only runs the reciprocal + rounding.
                    nc.scalar.activation(attnT[qrow:qrow + 64, p, :], pv[0:HD, :],
                                         mybir.ActivationFunctionType.Copy)
                    nc.scalar.activation(den_s[:, gg, :], pv[HD:HD + 1, :],
                                         mybir.ActivationFunctionType.Copy)
                else:
                    nc.vector.tensor_copy(attnT[qrow:qrow + 64, p, :], pv[0:HD, :])
                    nc.vector.tensor_copy(den_s[:, gg, :], pv[HD:HD + 1, :])
                nc.vector.reciprocal_approx_fast(
                    out=den_t[:, gg, :], in_=den_s[:, gg, :],
                )
                drow = 64 * (p % 2) + 32 * gg
                if (p, gg) == (KC - 1, G - 1):
                    # row 96 is not a legal matmul operand base; the final
                    # g1 reciprocal reuses row 32 (pair KC-3's slot, long
                    # consumed) so the tail broadcast can read at base 32.
                    drow = 32
                nc.vector.tensor_copy(den_r[drow:drow + 1, :], den_t[:, gg, :])

            # Per pair, the PE stream is: scores_g0, qproj fill, scores_g1,
            # rb fill, PV_g0, PV_g1 — both heads' exp+mask latency hides
            # behind the projection/score matmuls, so the PE never waits on
            # the scalar/gpsimd softmax chain and its clock stays ramped.
            q_proj(0)
            for p in range(KC):
                psc0 = emit_scores(p, 0)
                pt0 = emit_softmax(psc0)
                if p < KC - 1:
                    q_proj(p + 1)
                psc1 = emit_scores(p, 1)
                if p > 0:
                    # previous pair's denominators are ready by now (their
                    # reciprocals overlapped this pair's score matmuls)
                    norm_pair(p - 1)
                pt1 = emit_softmax(psc1)
                emit_pv(p, 0, pt0)
                emit_pv(p, 1, pt1)

            # last pair: normalize per group so only the g1 reciprocal is on
            # the tail's critical path. The g1 broadcast matmul is emitted
            # inside the first out-proj chain (right before the matmul that
            # consumes it) so the chain's earlier pairs hide the latency.
            b7 = 64 * ((KC - 1) % 2)
            psrb = psA.tile([128, 2, 512], f32, tag="ps", name="psrb7")
            nc.tensor.matmul(psrb[0:64, 0, :], r(selT_sb[b7:b7 + 1, 0:64]),
                             r(den_r[b7:b7 + 1, :]), start=True, stop=True)
            nc.vector.tensor_mul(attnT[0:64, KC - 1, :], attnT[0:64, KC - 1, :],
                                 psrb[0:64, 0, :])

            if dbg:
                nc.sync.dma_start(out=dbg_t["dbg_denf"][:, :], in_=den_f[:, :])
                nc.gpsimd.dma_start(out=dbg_t["dbg_denr"][:, :], in_=den_r[:, :])
                nc.gpsimd.dma_start(out=dbg_t["dbg_attnT"][:, :, :], in_=attnT[:, :, :])

            # ---- output projection ----
            for do in range(KC):
                ps = psA.tile([128, 2, 512], f32, tag="ps")
                out = ps[:, 0, :]
                for p in range(KC):
                    if do == 0 and p == KC - 1:
                        nc.tensor.matmul(
                            psrb[0:64, 1, :], r(selT_sb[32:33, 64:128]),
                            r(den_r[32:33, :]),
                            start=True, stop=True, skip_group_check=True,
                        )
                        nc.vector.tensor_mul(attnT[64:128, KC - 1, :],
                                             attnT[64:128, KC - 1, :],
                                             psrb[0:64, 1, :])
                    nc.tensor.matmul(
                        out, r(wo_sb[:, p, do * 128:(do + 1) * 128]),
                        r(attnT[:, p, :]),
                        start=(p == 0), stop=(p == KC - 1),
                        skip_group_check=(do == 0),
                    )
                yt = yst.tile([128, SQ], f32, tag="yt")
                nc.scalar.activation(yt, out, mybir.ActivationFunctionType.Identity,
                                     bias=bo_sb[:, do:do + 1])
                eng = nc.sync if do % 2 == 0 else nc.scalar
                eng.dma_start(out=yT[do * 128:(do + 1) * 128, :], in_=yt[:, :])

    nc.finalize()
    return nc


def get_program():
    if "nc" not in _CACHE:
        _CACHE["nc"] = _build_program()
    return _CACHE["nc"]


def _pack(w):
    """[1024, C] weight -> SBUF image [128 partitions, KC chunks, C]."""
    return np.ascontiguousarray(w.reshape(KC, 128, -1).transpose(1, 0, 2))


def make_in_maps(x, Wq, bq, Wk, bk, Wv, bv, Wo, bo):
    """Host-side sharding: per-core input dicts."""
    x = np.ascontiguousarray(np.asarray(x, np.float32))
    wkv_aug = np.concatenate([np.asarray(bk, np.float32), np.asarray(bv, np.float32)])
    # head permutation: device column-block p holds [head p | head p+8]
    # (so each q dd-block pairs a group-0 head with a group-1 head at
    # matching base partitions). perm maps device attn-dim -> original dim.
    perm = np.empty(DIM, np.int64)
    for p in range(8):
        perm[128 * p:128 * p + 64] = np.arange(64 * p, 64 * p + 64)
        perm[128 * p + 64:128 * p + 128] = np.arange(64 * (p + 8), 64 * (p + 8) + 64)
    wq_p = np.asarray(Wq, np.float32)[:, perm]
    # select weight: pair p (b0 = 64*(p%2)) broadcasts den_r row b0 to
    # partitions 0:64 and row b0+32 to partitions 64:128.
    selT = np.zeros((128, 128), np.float32)
    selT[0, :64] = 1.0
    selT[32, 64:] = 1.0
    selT[64, :64] = 1.0
    selT[96, 64:] = 1.0
    common = {
        "wqd0": _pack(wq_p[:, 0:128]),
        "wq0": _pack(wq_p[:, 128:512]),
        "wq1": _pack(wq_p[:, 512:1024]),
        "wkp": _pack(np.asarray(Wk, np.float32)),
        "wvp": _pack(np.asarray(Wv, np.float32)),
        "wkv_aug": np.ascontiguousarray(wkv_aug.reshape(1, 2 * G * HD)),
        "selT": selT,
        "wo": _pack(np.asarray(Wo, np.float32)[perm, :]),
        "bq": np.ascontiguousarray(np.asarray(bq, np.float32)[perm].reshape(DIM, 1)),
        "bo": np.ascontiguousarray(np.asarray(bo, np.float32).reshape(DIM, 1)),
        "ident": np.eye(128, dtype=np.float32),
        "ones2": np.ones((128, G), np.float32),
    }
    in_maps = []
    for c in range(NCORES):
        b, t = divmod(c, NCORES // BATCH)
        s0 = SQ * t
        xa = np.zeros((SK, DIM + 1), np.float32)
        lo, hi = max(0, s0 - HALF), min(SEQ, s0 + SQ + HALF)
        xa[lo - (s0 - HALF):hi - (s0 - HALF), :DIM] = x[b, lo:hi]
        xa[lo - (s0 - HALF):hi - (s0 - HALF), DIM] = 1.0
        xaT = xa.T  # [1025, 768]
        in_maps.append({
            "xaTp": _pack(xaT[0:DIM, :]),
            "xaug": np.ascontiguousarray(xaT[DIM:DIM + 1, :]),
            **common,
        })
    return in_maps


def assemble_output(results):
    y = np.empty((BATCH, SEQ, DIM), np.float32)
    for c in range(NCORES):
        b, t = divmod(c, NCORES // BATCH)
        y[b, SQ * t:SQ * (t + 1), :] = results[c]["yT"].T
    return y


def kernel(**inputs):
    from concourse.bass_utils import run_bass_kernel_spmd

    nc = get_program()
    in_maps = make_in_maps(**inputs)
    last_err = None
    for _ in range(3):  # retry: transient NRT device wedges recover on rerun
        try:
            res = run_bass_kernel_spmd(nc, in_maps, list(range(NCORES)))
            return assemble_output(res.results)
        except Exception as e:  # noqa: BLE001
            last_err = e
    raise last_err


if __name__ == "__main__":
    import reference

    inputs = reference.setup_inputs()
    expected = np.asarray(reference.reference(**inputs))
    actual = kernel(**{k: np.asarray(v) for k, v in inputs.items()})
    rel = np.linalg.norm(actual - expected) / np.linalg.norm(expected)
    print("rel err:", rel)


# revision 43
# speedup vs baseline: 1.1962x; 1.0193x over previous
"""GQA sliding-window attention (training path, no causal mask, no 1/sqrt(d)
scaling) on 8 Trainium2 NeuronCores.

Reference semantics (see original nn.Module):
  q = x@Wq+bq [b,s,16,64]; k,v = x@Wk+bk / x@Wv+bv [b,s,2,64]
  k,v zero-padded by 128 on both sides of s; query i attends padded
  positions [i, i+256) (i.e. global [i-128, i+128)); padded positions
  contribute score 0 (exp->1) and value 0. out = attn @ Wo + bo.

Sharding: batch x sequence. 8 shards = 2 batches x 4 chunks of 512 query
rows. Each core receives x^T for its 512 rows plus a 128-row halo on each
side (zero rows outside [0, 2048)), with an appended 0/1 validity row so
that K/V bias is only added at in-range positions (k = x@Wk + valid*bk).
Host gathers/concatenates per-core outputs; no collectives.

Per-core dataflow (all fp32; matmuls issued as float32r):
  xT -> qT [dk, s] / kT / vT projections (PE, contraction over 1024)
  vT transposed back to V [w, dk] via PE transpose; a ones-column is
  appended to V so each PV matmul also produces the softmax denominator.
  Scores computed transposed, S^T[w, q] = kT^T qT, per 128-wide kv chunk
  (6 chunks cover the 768 halo), q-window 384 per chunk.
  exp on ScalarE (one batched op per 3 chunks), band masking via
  GPSIMD affine_select triangles, PV accumulated over chunks into a
  [65, 512] PSUM tile (row 64 = denominator). Normalization is per head
  pair: 1/den via the fast DVE reciprocal straight off the PV PSUM row,
  broadcast across partitions with a tiny select matmul, multiplied into
  attnT right away so the Wo projection streams without a tail stall.

Input DMAs are consolidated into a handful of large strided transfers
issued in compute-priority order (wkv, x, wq halves, wo) on one queue so
the PE starts as soon as the first megabyte lands instead of after the
whole 15MB.
"""

import numpy as np

DIM = 1024
NH = 16  # query heads
G = 2  # kv heads
HD = 64  # head dim
W = 256  # window
HALF = 128
BATCH, SEQ = 2, 2048
NCORES = 8
SQ = 512  # query rows per core
SK = SQ + 2 * HALF  # 768 kv halo rows per core
KC = DIM // 128  # 8 contraction chunks
NJ = SK // 128  # 6 kv chunks
SP = 384  # score q-window width per kv chunk

# chunk j's PV accumulation window [lo, hi) in local q coords (exact: the
# bf16 pt/vt operands run 1 cycle/row at any N, so no N>=256 widening).
PV_WIN = {0: (0, 128), 1: (0, 256), 2: (0, 384), 3: (128, 512), 4: (256, 512), 5: (384, 512)}
WS = {j: (0 if j < 3 else 128) for j in range(NJ)}
# PV issue order: j1 [0,256) and j4 [256,512) partition the PSUM zero
# region exactly, so every byte is written once before any accumulation
# (has_written zero-region semantics); stop on the last.
PV_ORDER = [1, 4, 0, 2, 3, 5]

_CACHE = {}


def _build_program(dbg=False):
    import concourse.bass as bass
    import concourse.mybir as mybir
    import concourse.tile as tile
    from concourse import bacc

    f32 = mybir.dt.float32
    f32r = mybir.dt.float32r

    nc = bacc.Bacc("TRN2", target_bir_lowering=False, debug=False, num_devices=NCORES)
    dbg_t = {}
    if dbg:
        for name, shape, dt_ in [
            ("dbg_denf", [128, SQ], f32), ("dbg_denr", [128, SQ], f32),
            ("dbg_attnT", [128, KC, SQ], f32),
        ]:
            dbg_t[name] = nc.declare_dram_parameter(name, shape, dt_, isOutput=True)

    # All big inputs are host-packed to the exact SBUF image ([partition,
    # chunk, col], contiguous) so each load is one fully-contiguous DMA:
    # 128 descriptors with multi-KB lines instead of thousands of 1KB ones.
    xaTp = nc.declare_dram_parameter("xaTp", [128, KC, SK], f32r, isOutput=False)
    xaug_d = nc.declare_dram_parameter("xaug", [1, SK], f32r, isOutput=False)
    wqd0 = nc.declare_dram_parameter("wqd0", [128, KC, 128], f32r, isOutput=False)
    wq0 = nc.declare_dram_parameter("wq0", [128, KC, 384], f32r, isOutput=False)
    wq1 = nc.declare_dram_parameter("wq1", [128, KC, 512], f32r, isOutput=False)
    wkp = nc.declare_dram_parameter("wkp", [128, KC, G * HD], f32r, isOutput=False)
    wvp = nc.declare_dram_parameter("wvp", [128, KC, G * HD], f32r, isOutput=False)
    wkv_aug_d = nc.declare_dram_parameter("wkv_aug", [1, 2 * G * HD], f32r, isOutput=False)
    selT = nc.declare_dram_parameter("selT", [128, 128], f32r, isOutput=False)
    wo = nc.declare_dram_parameter("wo", [128, KC, DIM], f32r, isOutput=False)
    bq = nc.declare_dram_parameter("bq", [DIM, 1], f32, isOutput=False)
    bo = nc.declare_dram_parameter("bo", [DIM, 1], f32, isOutput=False)
    identD = nc.declare_dram_parameter("ident", [128, 128], f32r, isOutput=False)
    ones2 = nc.declare_dram_parameter("ones2", [128, G], f32r, isOutput=False)
    yT = nc.declare_dram_parameter("yT", [DIM, SQ], f32, isOutput=True)

    def r(ap):
        return ap

    with tile.TileContext(nc) as tc:
        with (
            nc.allow_low_precision("fp32r (tf32) matmul inputs; accumulation stays fp32"),
            tc.tile_pool(name="wts", bufs=1) as wts,
            tc.tile_pool(name="sb", bufs=1) as sb,
            tc.tile_pool(name="pt", bufs=3) as ptp,
            tc.tile_pool(name="yst", bufs=2) as yst,
            tc.tile_pool(name="psA", bufs=3, space="PSUM") as psA,
            tc.tile_pool(name="pvP", bufs=2, space="PSUM") as pvP,
        ):
            # ---- big loads: one fully-contiguous DMA each on the sync
            # queue, issued in compute-priority order so completion is
            # progressive: wk -> x (2-chunk pieces, so the K projection
            # trails the stream tightly) -> wv -> wq halves -> wo.
            wk_sb = wts.tile([128, KC, G * HD], f32r, tag="wk")
            nc.sync.dma_start(out=wk_sb[:, :, :], in_=wkp[:, :, :])
            xT_sb = wts.tile([128, KC, SK], f32r, tag="xT")
            for (a, b) in ((0, 1), (1, 2), (2, 4), (4, 6), (6, 8)):
                nc.sync.dma_start(out=xT_sb[:, a:b, :], in_=xaTp[:, a:b, :])
            wv_sb = wts.tile([128, KC, G * HD], f32r, tag="wv")
            nc.sync.dma_start(out=wv_sb[:, :, :], in_=wvp[:, :, :])
            # wq split so q_proj(0) only waits on its own 0.5MB column block
            wqd0_sb = wts.tile([128, KC, 128], f32r, tag="wqd0")
            nc.sync.dma_start(out=wqd0_sb[:, :, :], in_=wqd0[:, :, :])
            wq0_sb = wts.tile([128, KC, 384], f32r, tag="wq0")
            wq1_sb = wts.tile([128, KC, 512], f32r, tag="wq1")
            nc.sync.dma_start(out=wq0_sb[:, :, :], in_=wq0[:, :, :])
            nc.sync.dma_start(out=wq1_sb[:, :, :], in_=wq1[:, :, :])
            wo_sb = wts.tile([128, KC, DIM], f32r, tag="wo")
            nc.sync.dma_start(out=wo_sb[:, :, :], in_=wo[:, :, :])

            # ---- small constants, split over the gpsimd SWDGE and scalar
            # queues (issued after the big streams so their descriptors
            # don't clog the rings ahead of the weights)
            xaug = wts.tile([1, SK], f32r, tag="xaug")
            nc.gpsimd.dma_start(out=xaug[:, :], in_=xaug_d[:, :])
            wkv_aug = wts.tile([1, 2 * G * HD], f32r, tag="wkvaug")
            nc.gpsimd.dma_start(out=wkv_aug[:, :], in_=wkv_aug_d[:, :])
            bq_sb = wts.tile([128, KC], f32, tag="bq")
            bo_sb = wts.tile([128, KC], f32, tag="bo")
            nc.gpsimd.dma_start(
                out=bq_sb[:, :], in_=bq.rearrange("(a p) c -> p (a c)", p=128))
            nc.scalar.dma_start(
                out=bo_sb[:, :], in_=bo.rearrange("(a p) c -> p (a c)", p=128))
            ident = wts.tile([128, 128], f32r, tag="ident")
            nc.scalar.dma_start(out=ident[:, :], in_=identD[:, :])
            ones_sb = wts.tile([128, G], f32r, tag="ones")
            nc.scalar.dma_start(out=ones_sb[:, :], in_=ones2[:, :])
            selT_sb = wts.tile([128, 128], f32r, tag="selT")
            nc.scalar.dma_start(out=selT_sb[:, :], in_=selT[:, :])

            # ---- persistent intermediates ----
            qT_sb = sb.tile([128, KC, SQ], f32r, tag="qT")     # [dk(2 heads), dd, q]
            kT_sb = sb.tile([128, SK], f32r, tag="kT")         # [dk(2 groups), w]
            vT_sb = sb.tile([128, SK], f32r, tag="vT")
            bf16 = mybir.dt.bfloat16
            vt_t = [
                sb.tile([128, G, HD + 1], bf16, tag=f"vt{j}", name=f"vt{j}")
                for j in range(NJ)
            ]
            attnT = sb.tile([128, KC, SQ], f32r, tag="attnT")  # [dk(2 heads), pair, q]
            # reciprocal denominators. The custom-DVE fast reciprocal only
            # works from/to SBUF at base partition 0 on hardware (CoreSim
            # idealizes other bases): copy the PSUM den row -> den_s (base
            # 0), approx-reciprocal -> den_t (base 0). A gpsimd
            # partition_broadcast then fans each head's row across 64
            # partitions (rb tiles, also written at base 0) for the
            # in-place normalization multiply.
            den_s = sb.tile([1, G, SQ], f32, tag="dens")
            den_t = sb.tile([1, G, SQ], f32, tag="dent")
            den_f = sb.tile([128, SQ], f32, tag="denf")
            den_r = sb.tile([128, SQ], f32r, tag="denr")
            # memset can't target f32r; bounce the 1.0 fill through den_f
            nc.vector.memset(den_f[:, :], 1.0)
            nc.vector.tensor_copy(den_r[:, :], den_f[:, :])

            # ---- K/V projections over the full 768 halo (+ aug bias row) ----
            for (c0, wsb, dst) in ((0, wk_sb, kT_sb), (G * HD, wv_sb, vT_sb)):
                for h2 in range(2):
                    ps = psA.tile([128, 2, 512], f32, tag="ps")
                    out = ps[:, 0, 0:SP]
                    sl = slice(h2 * SP, (h2 + 1) * SP)
                    for kc in range(KC):
                        nc.tensor.matmul(
                            out, r(wsb[:, kc, :]), r(xT_sb[:, kc, sl]),
                            start=(kc == 0), stop=False,
                        )
                    nc.tensor.matmul(out, r(wkv_aug[:, c0:c0 + G * HD]), r(xaug[:, sl]),
                                     start=False, stop=True)
                    nc.vector.tensor_copy(dst[:, sl], out)

            # ---- V back to natural layout [w, dk], ones column appended ----
            for j in range(NJ):
                ps = psA.tile([128, 2, 512], f32r, tag="ps", name=f"pstr{j}")
                out = ps[:, 0, 0:128]
                nc.tensor.transpose(out, vT_sb[:, j * 128:(j + 1) * 128], ident)
                nc.vector.tensor_copy(
                    vt_t[j][:, :, 0:HD],
                    out.rearrange("p (g d) -> p g d", g=G),
                )
                nc.vector.tensor_copy(vt_t[j][:, :, HD:HD + 1], ones_sb[:, :])

            # ---- attention per head, normalization per pair ----
            # Host permutes Wq columns so q dd-block p holds head p (group 0)
            # in rows 0:64 and head p+8 (group 1) in rows 64:128 — score
            # matmul operands then share a base partition with kT's groups.
            def q_proj(dd):
                # Q projection block dd: qT[dd] = (Wq^T x^T)[dd] + bq.
                # Emitted just-in-time inside the attention loop so the PE
                # stream stays dense (512-wide projection matmuls fill the
                # gaps between attention groups and keep the clock un-gated).
                # Uses the 1-bank pvP pool so the score psum pool keeps all
                # three of its buffers for the psc tiles.
                ps = pvP.tile([128, 512], f32, tag="pv", name=f"psq{dd}")
                if dd == 0:
                    wsb, c0 = wqd0_sb, 0
                elif dd < 4:
                    wsb, c0 = wq0_sb, (dd - 1) * 128
                else:
                    wsb, c0 = wq1_sb, (dd - 4) * 128
                for kc in range(KC):
                    nc.tensor.matmul(
                        ps[:, :], r(wsb[:, kc, c0:c0 + 128]),
                        r(xT_sb[:, kc, HALF:HALF + SQ]),
                        start=(kc == 0), stop=(kc == KC - 1),
                    )
                nc.scalar.activation(
                    qT_sb[:, dd, :], ps[:, :], mybir.ActivationFunctionType.Identity,
                    bias=bq_sb[:, dd:dd + 1],
                )

            def norm_pair(p):
                # broadcast 1/den across partitions (rows 0:64 <- g0 head,
                # 64:128 <- g1 head) and normalize attnT[:, p, :] in place.
                b0 = 64 * (p % 2)
                ps = psA.tile([128, 2, 512], f32, tag="ps", name=f"psrb{p}")
                rb = ps[:, 0, :]
                nc.tensor.matmul(
                    rb, r(selT_sb[b0:b0 + 33, :]),
                    r(den_r[b0:b0 + 33, :]),
                    start=True, stop=True,
                )
                nc.vector.tensor_mul(attnT[:, p, :], attnT[:, p, :], rb)

            def emit_scores(p, gg):
                h = p + 8 * gg
                qT_h = qT_sb[64 * gg:64 * gg + 64, p, :]
                psc = [
                    psA.tile([128, 2, 512], f32, tag="ps", name=f"psc{h}_{i}")
                    for i in range(3)
                ]
                for j in range(NJ):
                    ws = WS[j]
                    nc.tensor.matmul(
                        psc[j // 2][:, j % 2, 0:SP],
                        r(kT_sb[64 * gg:64 * gg + 64, j * 128:(j + 1) * 128]),
                        r(qT_h[:, ws:ws + SP]),
                        start=True, stop=True,
                    )
                return psc

            def emit_softmax(psc):
                pt = ptp.tile([128, NJ, SP], bf16, tag="pt")
                # exp only the columns the PV windows read: thirds cover
                # chunk pairs (0,1): cols [0,256), (2,3): [0,384),
                # (4,5): [128,384). Emission order 0,2,1 matches PV_ORDER's
                # chunk consumption (j1 then j4) so the first PV matmuls
                # aren't stuck behind an exp they don't need.
                for c3 in (0, 2, 1):
                    elo, ehi = ((0, 256), (0, SP), (128, SP))[c3]
                    nc.scalar.activation(pt[:, 2 * c3:2 * c3 + 2, elo:ehi],
                                         psc[c3][:, :, elo:ehi],
                                         mybir.ActivationFunctionType.Exp)

                # band masking: keep iff 0 <= (128j + ww) - q < 256.
                # The upper-bound select on chunk 0 spans cols [0,256) (its
                # widened PV window): cols >= 128 fail the condition for
                # every partition and are filled with 0. Likewise the
                # lower-bound select on chunk 5. Chunks masked in PV_ORDER
                # so the serial gpsimd stream feeds the PV matmuls in their
                # issue order.
                for j in PV_ORDER:
                    lo, hi = PV_WIN[j]
                    ws = WS[j]
                    cb = lo
                    while cb < hi:
                        wdt = 128
                        c0 = cb - ws
                        region = pt[:, j, c0:c0 + wdt]
                        if cb > 128 * (j - 1):  # upper bound: q <= 128j + ww
                            nc.gpsimd.affine_select(
                                out=region, in_=region,
                                compare_op=mybir.AluOpType.is_ge, fill=0.0,
                                base=128 * j - cb, channel_multiplier=1,
                                pattern=[[-1, wdt]],
                            )
                        elif cb < 128 * (j - 1):  # lower: q > 128j + ww - 256
                            nc.gpsimd.affine_select(
                                out=region, in_=region,
                                compare_op=mybir.AluOpType.is_ge, fill=0.0,
                                base=cb - 128 * j + 255, channel_multiplier=-1,
                                pattern=[[1, wdt]],
                            )
                        cb += wdt
                return pt

            def emit_pv(p, gg, pt):
                qrow = 64 * gg
                pv = pvP.tile([128, 512], f32, tag="pv")
                for j in PV_ORDER:
                    lo, hi = PV_WIN[j]
                    ws = WS[j]
                    nc.tensor.matmul(
                        pv[0:HD + 1, lo:hi],
                        r(vt_t[j][:, gg, :]),
                        r(pt[:, j, lo - ws:hi - ws]),
                        start=(j == PV_ORDER[0]), stop=(j == PV_ORDER[-1]),
                    )
                if (p, gg) == (KC - 1, G - 1):
                    # last pair: the reciprocal chain is the tail's critical
                    # path; push the two copies to the (now idle) scalar
                    # engine so the DVE only runs the reciprocal + rounding.
                    nc.scalar.activation(attnT[qrow:qrow + 64, p, :], pv[0:HD, :],
                                         mybir.ActivationFunctionType.Copy)
                    nc.scalar.activation(den_s[:, gg, :], pv[HD:HD + 1, :],
                                         mybir.ActivationFunctionType.Copy)
                else:
                    nc.vector.tensor_copy(attnT[qrow:qrow + 64, p, :], pv[0:HD, :])
                    nc.vector.tensor_copy(den_s[:, gg, :], pv[HD:HD + 1, :])
                nc.vector.reciprocal_approx_fast(
                    out=den_t[:, gg, :], in_=den_s[:, gg, :],
                )
                drow = 64 * (p % 2) + 32 * gg
                if (p, gg) == (KC - 1, G - 1):
                    # row 96 is not a legal matmul operand base; the final
                    # g1 reciprocal reuses row 32 (pair KC-3's slot, long
                    # consumed) so the tail broadcast can read at base 32.
                    drow = 32
                nc.vector.tensor_copy(den_r[drow:drow + 1, :], den_t[:, gg, :])

            # Per pair, the PE stream is: scores_g0, qproj fill, scores_g1,
            # rb fill, PV_g0, PV_g1 — both heads' exp+mask latency hides
            # behind the projection/score matmuls, so the PE never waits on
            # the scalar/gpsimd softmax chain and its clock stays ramped.
            q_proj(0)
            for p in range(KC):
                psc0 = emit_scores(p, 0)
                pt0 = emit_softmax(psc0)
                if p < KC - 1:
                    q_proj(p + 1)
                psc1 = emit_scores(p, 1)
                if p > 0:
                    # previous pair's denominators are ready by now (their
                    # reciprocals overlapped this pair's score matmuls)
                    norm_pair(p - 1)
                pt1 = emit_softmax(psc1)
                emit_pv(p, 0, pt0)
                emit_pv(p, 1, pt1)

            # last pair: normalize per group so only the g1 reciprocal is on
            # the tail's critical path. The g1 broadcast matmul is emitted
            # inside the first out-proj chain (right before the matmul that
            # consumes it) so the chain's earlier pairs hide the latency.
            b7 = 64 * ((KC - 1) % 2)
            psrb = psA.tile([128, 2, 512], f32, tag="ps", name="psrb7")
            nc.tensor.matmul(psrb[0:64, 0, :], r(selT_sb[b7:b7 + 1, 0:64]),
                             r(den_r[b7:b7 + 1, :]), start=True, stop=True)
            nc.vector.tensor_mul(attnT[0:64, KC - 1, :], attnT[0:64, KC - 1, :],
                                 psrb[0:64, 0, :])

            if dbg:
                nc.sync.dma_start(out=dbg_t["dbg_denf"][:, :], in_=den_f[:, :])
                nc.gpsimd.dma_start(out=dbg_t["dbg_denr"][:, :], in_=den_r[:, :])
                nc.gpsimd.dma_start(out=dbg_t["dbg_attnT"][:, :, :], in_=attnT[:, :, :])

            # ---- output projection ----
            for do in range(KC):
                ps = psA.tile([128, 2, 512], f32, tag="ps")
                out = ps[:, 0, :]
                for p in range(KC):
                    if do == 0 and p == KC - 1:
                        nc.tensor.matmul(
                            psrb[0:64, 1, :], r(selT_sb[32:33, 64:128]),
                            r(den_r[32:33, :]),
                            start=True, stop=True, skip_group_check=True,
                        )
                        nc.vector.tensor_mul(attnT[64:128, KC - 1, :],
                                             attnT[64:128, KC - 1, :],
                                             psrb[0:64, 1, :])
                    nc.tensor.matmul(
                        out, r(wo_sb[:, p, do * 128:(do + 1) * 128]),
                        r(attnT[:, p, :]),
                        start=(p == 0), stop=(p == KC - 1),
                        skip_group_check=(do == 0),
                    )
                yt = yst.tile([128, SQ], f32, tag="yt")
                nc.scalar.activation(yt, out, mybir.ActivationFunctionType.Identity,
                                     bias=bo_sb[:, do:do + 1])
                eng = nc.sync if do % 2 == 0 else nc.scalar
                eng.dma_start(out=yT[do * 128:(do + 1) * 128, :], in_=yt[:, :])

    nc.finalize()
    return nc


def get_program():
    if "nc" not in _CACHE:
        _CACHE["nc"] = _build_program()
    return _CACHE["nc"]


def _pack(w):
    """[1024, C] weight -> SBUF image [128 partitions, KC chunks, C]."""
    return np.ascontiguousarray(w.reshape(KC, 128, -1).transpose(1, 0, 2))


def make_in_maps(x, Wq, bq, Wk, bk, Wv, bv, Wo, bo):
    """Host-side sharding: per-core input dicts."""
    x = np.ascontiguousarray(np.asarray(x, np.float32))
    wkv_aug = np.concatenate([np.asarray(bk, np.float32), np.asarray(bv, np.float32)])
    # head permutation: device column-block p holds [head p | head p+8]
    # (so each q dd-block pairs a group-0 head with a group-1 head at
    # matching base partitions). perm maps device attn-dim -> original dim.
    perm = np.empty(DIM, np.int64)
    for p in range(8):
        perm[128 * p:128 * p + 64] = np.arange(64 * p, 64 * p + 64)
        perm[128 * p + 64:128 * p + 128] = np.arange(64 * (p + 8), 64 * (p + 8) + 64)
    wq_p = np.asarray(Wq, np.float32)[:, perm]
    # select weight: pair p (b0 = 64*(p%2)) broadcasts den_r row b0 to
    # partitions 0:64 and row b0+32 to partitions 64:128.
    selT = np.zeros((128, 128), np.float32)
    selT[0, :64] = 1.0
    selT[32, 64:] = 1.0
    selT[64, :64] = 1.0
    selT[96, 64:] = 1.0
    common = {
        "wqd0": _pack(wq_p[:, 0:128]),
        "wq0": _pack(wq_p[:, 128:512]),
        "wq1": _pack(wq_p[:, 512:1024]),
        "wkp": _pack(np.asarray(Wk, np.float32)),
        "wvp": _pack(np.asarray(Wv, np.float32)),
        "wkv_aug": np.ascontiguousarray(wkv_aug.reshape(1, 2 * G * HD)),
        "selT": selT,
        "wo": _pack(np.asarray(Wo, np.float32)[perm, :]),
        "bq": np.ascontiguousarray(np.asarray(bq, np.float32)[perm].reshape(DIM, 1)),
        "bo": np.ascontiguousarray(np.asarray(bo, np.float32).reshape(DIM, 1)),
        "ident": np.eye(128, dtype=np.float32),
        "ones2": np.ones((128, G), np.float32),
    }
    in_maps = []
    for c in range(NCORES):
        b, t = divmod(c, NCORES // BATCH)
        s0 = SQ * t
        xa = np.zeros((SK, DIM + 1), np.float32)
        lo, hi = max(0, s0 - HALF), min(SEQ, s0 + SQ + HALF)
        xa[lo - (s0 - HALF):hi - (s0 - HALF), :DIM] = x[b, lo:hi]
        xa[lo - (s0 - HALF):hi - (s0 - HALF), DIM] = 1.0
        xaT = xa.T  # [1025, 768]
        in_maps.append({
            "xaTp": _pack(xaT[0:DIM, :]),
            "xaug": np.ascontiguousarray(xaT[DIM:DIM + 1, :]),
            **common,
        })
    return in_maps


def assemble_output(results):
    y = np.empty((BATCH, SEQ, DIM), np.float32)
    for c in range(NCORES):
        b, t = divmod(c, NCORES // BATCH)
        y[b, SQ * t:SQ * (t + 1), :] = results[c]["yT"].T
    return y


def kernel(**inputs):
    from concourse.bass_utils import run_bass_kernel_spmd

    nc = get_program()
    in_maps = make_in_maps(**inputs)
    last_err = None
    for _ in range(3):  # retry: transient NRT device wedges recover on rerun
        try:
            res = run_bass_kernel_spmd(nc, in_maps, list(range(NCORES)))
            return assemble_output(res.results)
        except Exception as e:  # noqa: BLE001
            last_err = e
    raise last_err


if __name__ == "__main__":
    import reference

    inputs = reference.setup_inputs()
    expected = np.asarray(reference.reference(**inputs))
    actual = kernel(**{k: np.asarray(v) for k, v in inputs.items()})
    rel = np.linalg.norm(actual - expected) / np.linalg.norm(expected)
    print("rel err:", rel)
